# revision 2
# baseline (speedup 1.0000x reference)
"""Trainium2 Bass kernel for nn_CustomLstm (D=2048, H=1024), 8-core tensor-parallel.

Sharding: all five weights/biases and outputs are sharded along the units (row)
dimension of W across 8 NeuronCores (256 rows each).  The (D,D) concat
activation is replicated; gate elementwise ops are local; ht is all-gathered
(fp8, one AllGather per 128-row block per 512-col chunk) so the final
w5 @ ht matmul + row softmax is local.

Precision plan (validated vs fp32 reference, worst rel err ~1.0e-2):
  - sigmoid gates w1/w2/w4 and w5: fp8e4m3 DoubleRow matmuls (2 k-chunks per
    instruction, 1.5x measured PE throughput).  x (concat) and ht are scaled
    x64 before the fp8 cast so values stay in e4m3's normal range; the 1/64
    is folded into the activation/exp scale.  b1/b2/b4 ride fp8 (x64).
  - tanh gate w3 (slope 1 passes matmul error straight through): fp16.
  - b3/b5 fp16, cPrev fp16, outputs written fp16 and upcast on host.
"""

import numpy as np
import ml_dtypes

import concourse.bass as bass
import concourse.bacc as bacc
import concourse.mybir as mybir
import concourse.tile as tile
import concourse.bass_utils as bass_utils

BF16 = ml_dtypes.bfloat16
FP8 = ml_dtypes.float8_e4m3
F16 = np.float16

D = 2048          # units == input dim of each weight matrix
N_CORES = 8
R = D // N_CORES  # 256 rows per core
PK = D // 128     # 16 contraction chunks of 128
NN = 4            # 4 column chunks of 512
NCOL = D // NN    # 512
NM = R // 128     # 2 row chunks of 128
S = 64.0          # fp8 input scale

_CACHE = None


def _build(reps=1, single=False, fake_ag=False):
    nc = bacc.Bacc("TRN2", target_bir_lowering=False, debug=False,
                   num_devices=1 if single else N_CORES)
    f32 = mybir.dt.float32
    fp16 = mybir.dt.float16
    fp8 = mybir.dt.float8e4
    AF = mybir.ActivationFunctionType
    DR = mybir.MatmulPerfMode.DoubleRow

    x8_in = nc.dram_tensor("x8", [D, D], fp8, kind="ExternalInput").ap()
    x16_in = nc.dram_tensor("x16", [D, D], fp16, kind="ExternalInput").ap()
    # gate weights transposed: [D(contraction), R]; gate order w1,w2,w3,w4
    wg_in = [nc.dram_tensor(f"w{g}t", [D, R],
                            fp16 if g == 3 else fp8,
                            kind="ExternalInput").ap() for g in range(1, 5)]
    w5_in = nc.dram_tensor("w5t", [D, R], fp8, kind="ExternalInput").ap()
    bg_in = [nc.dram_tensor(f"b{g}", [R, D],
                            fp16 if g == 3 else fp8,
                            kind="ExternalInput").ap() for g in range(1, 5)]
    b5_in = nc.dram_tensor("b5", [R, D], fp16, kind="ExternalInput").ap()
    cprev_in = nc.dram_tensor("cprev", [R, D], fp16, kind="ExternalInput").ap()

    ct_o = nc.dram_tensor("ct_o", [R, D], fp16, kind="ExternalOutput").ap()
    ht_o = nc.dram_tensor("ht_o", [R, D], fp16, kind="ExternalOutput").ap()
    yt_o = nc.dram_tensor("yt_o", [R, D], fp16, kind="ExternalOutput").ap()

    rg = [list(range(N_CORES))]
    GFN = [AF.Sigmoid, AF.Sigmoid, AF.Tanh, AF.Sigmoid]
    GSC = [1.0 / S, 1.0 / S, 1.0, 1.0 / S]   # activation input scale per gate

    with tile.TileContext(nc) as tc:
        with (
            tc.tile_pool(name="wpool", bufs=1) as wpool,
            tc.tile_pool(name="xpool", bufs=2) as xpool,
            tc.tile_pool(name="bpool", bufs=1) as bpool,
            tc.tile_pool(name="hpool", bufs=2) as hpool,
            tc.tile_pool(name="gpool", bufs=1) as gpool,
            tc.tile_pool(name="epool", bufs=1) as epool,
            tc.tile_pool(name="spool", bufs=2) as spool,
            tc.tile_pool(name="ypool", bufs=4) as ypool,
            tc.tile_pool(name="psum", bufs=1, space="PSUM") as pp,
            tc.tile_pool(name="dram", bufs=1, space="DRAM") as dram,
        ):
            for rep in range(reps):
                # ---- resident loads (ACT queue), first-use order ----
                w_sb = [wpool.tile([128, PK * R], fp16 if g == 2 else fp8,
                                   name=f"w{g}sb", tag=f"w{g}sb")
                        for g in range(4)]
                w5_sb = wpool.tile([128, PK * R], fp8, name="w5sb", tag="w5sb")
                b_sb = [bpool.tile([128, NM * D], fp16 if g == 2 else fp8,
                                   name=f"b{g}sb", tag=f"b{g}sb")
                        for g in range(4)]
                b5_sb = bpool.tile([128, NM * D], fp16, name="b5sb", tag="b5sb")
                cp_sb = bpool.tile([128, NM * D], fp16, name="cpsb", tag="cpsb")

                def load_w(dst, src):
                    nc.scalar.dma_start(
                        dst[:].rearrange("p (k m) -> p k m", m=R),
                        src.rearrange("(k p) m -> p k m", p=128))

                def load_b(dst, src):
                    nc.scalar.dma_start(
                        dst[:].rearrange("p (m c) -> p m c", c=D),
                        src.rearrange("(m p) c -> p m c", p=128))

                load_w(w_sb[0], wg_in[0])
                load_w(w_sb[1], wg_in[1])
                load_b(b_sb[0], bg_in[0])
                load_b(b_sb[1], bg_in[1])
                load_w(w_sb[2], wg_in[2])
                load_b(b_sb[2], bg_in[2])
                load_w(w_sb[3], wg_in[3])
                load_b(b_sb[3], bg_in[3])
                load_b(cp_sb, cprev_in)
                load_w(w5_sb, w5_in)
                load_b(b5_sb, b5_in)

                w3d = [w_sb[g][:].rearrange("p (k m) -> p k m", m=R)
                       for g in range(4)]

                ag_out = [[None, None] for _ in range(NN)]
                sms = [[None] * NN for _ in range(NM)]
                exs = [epool.tile([128, D], fp16, name=f"ex{m}", tag=f"ex{m}")
                       for m in range(NM)]

                # ---- phase A: gates, ct, ht; AllGather ht per (chunk, m) ----
                for n in range(NN):
                    csl = slice(n * NCOL, (n + 1) * NCOL)
                    x8_sb = xpool.tile([128, PK * NCOL], fp8, name=f"x8_{n}",
                                       tag="x8")
                    nc.sync.dma_start(
                        x8_sb[:].rearrange("p (k c) -> p k c", c=NCOL),
                        x8_in[:, csl].rearrange("(k p) c -> p k c", p=128))
                    x16_sb = xpool.tile([128, PK * NCOL], fp16, name=f"x16_{n}",
                                        tag="x16")
                    nc.gpsimd.dma_start(
                        x16_sb[:].rearrange("p (k c) -> p k c", c=NCOL),
                        x16_in[:, csl].rearrange("(k p) c -> p k c", p=128))
                    x8d = x8_sb[:].rearrange("p (k c) -> p k c", c=NCOL)

                    ctt = gpool.tile([128, NM * NCOL], fp16, name="ctt",
                                     tag="ctt", bufs=2)
                    htt = gpool.tile([128, NM * NCOL], fp16, name="htt",
                                     tag="htt", bufs=2)

                    for m in range(NM):
                        ps = [None] * 4
                        for g in range(4):   # w1, w2, w3, w4
                            p = pp.tile([128, NCOL], f32, name=f"ps{g}",
                                        tag=f"ps{g}")
                            if g == 2:
                                for k in range(PK):
                                    nc.tensor.matmul(
                                        p[:],
                                        w_sb[2][:, k * R + m * 128:
                                                k * R + (m + 1) * 128],
                                        x16_sb[:, k * NCOL:(k + 1) * NCOL],
                                        start=(k == 0), stop=(k == PK - 1))
                            else:
                                for j in range(PK // 2):
                                    nc.tensor.matmul(
                                        p[:],
                                        w3d[g][:, 2 * j:2 * j + 2,
                                               m * 128:(m + 1) * 128],
                                        x8d[:, 2 * j:2 * j + 2, :],
                                        start=(j == 0), stop=(j == PK // 2 - 1),
                                        perf_mode=DR)
                            ps[g] = p

                        acts = []
                        for g in range(4):
                            bsl = b_sb[g][:, m * D + n * NCOL:
                                          m * D + (n + 1) * NCOL]
                            nc.vector.tensor_add(ps[g][:], ps[g][:], bsl)
                            act = gpool.tile([128, NCOL], fp16, name=f"act{g}",
                                             tag=f"act{g}")
                            nc.scalar.activation(act[:], ps[g][:], GFN[g],
                                                 scale=GSC[g])
                            acts.append(act)

                        msl = slice(m * NCOL, (m + 1) * NCOL)
                        cpsl = cp_sb[:, m * D + n * NCOL:m * D + (n + 1) * NCOL]
                        t1 = gpool.tile([128, NCOL], fp16, name="t1", tag="t1")
                        nc.vector.tensor_mul(t1[:], acts[0][:], cpsl)
                        t2 = gpool.tile([128, NCOL], fp16, name="t2", tag="t2")
                        nc.vector.tensor_mul(t2[:], acts[1][:], acts[2][:])
                        nc.vector.tensor_add(ctt[:, msl], t1[:], t2[:])

                        th = gpool.tile([128, NCOL], fp16, name="th", tag="th")
                        nc.scalar.activation(th[:], ctt[:, msl], AF.Tanh)
                        nc.vector.tensor_mul(htt[:, msl], acts[3][:], th[:])
                        htb = gpool.tile([128, NCOL], fp8, name="htb",
                                         tag="htb", bufs=2)
                        nc.scalar.activation(htb[:], htt[:, msl], AF.Copy,
                                             scale=S)

                        agi = dram.tile([128, NCOL], fp8, name=f"agi{n}{m}",
                                        tag=f"agi{n}{m}")
                        nc.scalar.dma_start(agi[:], htb[:])
                        asp = "Local" if (single or fake_ag) else "Shared"
                        ago = dram.tile([N_CORES * 128, NCOL], fp8,
                                        name=f"ago{n}{m}", tag=f"ago{n}{m}",
                                        addr_space=asp)
                        if single or fake_ag:
                            for blk in range(N_CORES):
                                nc.gpsimd.dma_start(
                                    ago[blk * 128:(blk + 1) * 128, :], agi[:])
                        else:
                            nc.gpsimd.collective_compute(
                                "AllGather", mybir.AluOpType.bypass,
                                replica_groups=rg,
                                ins=[agi.opt()], outs=[ago.opt()])
                        ag_out[n][m] = ago

                    nc.gpsimd.dma_start(
                        ct_o[:, csl].rearrange("(m p) c -> p m c", p=128),
                        ctt[:].rearrange("p (m c) -> p m c", c=NCOL))
                    nc.gpsimd.dma_start(
                        ht_o[:, csl].rearrange("(m p) c -> p m c", p=128),
                        htt[:].rearrange("p (m c) -> p m c", c=NCOL))

                # ---- phase C: z5 = w5 @ ht + b5, row softmax ----
                w53 = w5_sb[:].rearrange("p (k m) -> p k m", m=R)
                for n in range(NN):
                    csl = slice(n * NCOL, (n + 1) * NCOL)
                    hh = []   # two half-contraction tiles: rows of ag m0, m1
                    for a in range(2):
                        h = hpool.tile([128, (PK // 2) * NCOL], fp8,
                                       name=f"h{n}{a}", tag=f"h{a}")
                        nc.sync.dma_start(
                            h[:].rearrange("p (k c) -> p k c", c=NCOL),
                            ag_out[n][a].rearrange("(k p) c -> p k c", p=128))
                        hh.append(h[:].rearrange("p (k c) -> p k c", c=NCOL))

                    p5 = [pp.tile([128, NCOL], f32, name=f"ps5{m}", tag="ps5",
                                  bufs=2) for m in range(NM)]
                    for a in range(2):       # interleave m so half-a work
                        for m in range(NM):  # starts before half-b arrives
                            for j in range(PK // 4):
                                nc.tensor.matmul(
                                    p5[m][:],
                                    w53[:, 8 * a + 2 * j:8 * a + 2 * j + 2,
                                        m * 128:(m + 1) * 128],
                                    hh[a][:, 2 * j:2 * j + 2, :],
                                    start=(a == 0 and j == 0),
                                    stop=(a == 1 and j == PK // 4 - 1),
                                    perf_mode=DR)
                    for m in range(NM):
                        b5sl = b5_sb[:, m * D + n * NCOL:m * D + (n + 1) * NCOL]
                        nc.vector.tensor_add(p5[m][:], p5[m][:], b5sl)
                        sm_t = spool.tile([128, 1], f32, name=f"sm{m}{n}",
                                          tag=f"sm{m}{n}")
                        nc.scalar.activation(exs[m][:, csl], p5[m][:], AF.Exp,
                                             scale=1.0 / S, accum_out=sm_t[:])
                        sms[m][n] = sm_t

                # ---- softmax normalize + yt out ----
                for m in range(NM):
                    s01 = spool.tile([128, 1], f32, name="s01", tag="s01")
                    nc.vector.tensor_add(s01[:], sms[m][0][:], sms[m][1][:])
                    s23 = spool.tile([128, 1], f32, name="s23", tag="s23")
                    nc.vector.tensor_add(s23[:], sms[m][2][:], sms[m][3][:])
                    smt = spool.tile([128, 1], f32, name="smt", tag="smt")
                    nc.vector.tensor_add(smt[:], s01[:], s23[:])
                    rs = spool.tile([128, 1], f32, name="rs", tag="rs")
                    nc.vector.reciprocal(rs[:], smt[:])
                    for j in range(NN):
                        jsl = slice(j * NCOL, (j + 1) * NCOL)
                        yp = ypool.tile([128, NCOL], fp16, name="yp", tag="yp")
                        if j % 2 == 0:
                            nc.vector.tensor_scalar_mul(yp[:], exs[m][:, jsl],
                                                        rs[:])
                        else:
                            nc.scalar.activation(yp[:], exs[m][:, jsl],
                                                 AF.Copy, scale=rs[:])
                        nc.sync.dma_start(yt_o[m * 128:(m + 1) * 128, jsl],
                                          yp[:])

    nc.compile()
    return nc


_RUNNER = None


def _build_runner(nc):
    """Cached jit-compiled SPMD executor mirroring run_bass_kernel_spmd's
    axon/PJRT path, so repeat kernel() calls skip retracing."""
    import jax
    from jax.sharding import Mesh, PartitionSpec, NamedSharding
    from jax.experimental.shard_map import shard_map
    from concourse.bass2jax import (_bass_exec_p, install_neuronx_cc_hook,
                                    partition_id_tensor)

    install_neuronx_cc_hook()
    partition_name = (nc.partition_id_tensor.name
                      if nc.partition_id_tensor else None)
    in_names, out_names, out_avals = [], [], []
    for alloc in nc.m.functions[0].allocations:
        if not isinstance(alloc, mybir.MemoryLocationSet):
            continue
        name = alloc.memorylocations[0].name
        if alloc.kind == "ExternalInput":
            if name != partition_name:
                in_names.append(name)
        elif alloc.kind == "ExternalOutput":
            out_names.append(name)
            out_avals.append(jax.core.ShapedArray(
                tuple(alloc.tensor_shape), mybir.dt.np(alloc.dtype)))
    n_params, n_outs = len(in_names), len(out_names)
    all_in = tuple(in_names + out_names
                   + ([partition_name] if partition_name else []))

    def _body(*args):
        operands = list(args)
        if partition_name is not None:
            operands.append(partition_id_tensor())
        return tuple(_bass_exec_p.bind(
            *operands, out_avals=tuple(out_avals), in_names=all_in,
            out_names=tuple(out_names), lowering_input_output_aliases=(),
            sim_require_finite=True, sim_require_nnan=True, nc=nc))

    devices = jax.devices()[:N_CORES]
    mesh = Mesh(np.asarray(devices), ("core",))
    specs = (PartitionSpec("core"),) * (n_params + n_outs)
    fn = jax.jit(
        shard_map(_body, mesh=mesh, in_specs=specs,
                  out_specs=(PartitionSpec("core"),) * n_outs,
                  check_rep=False),
        donate_argnums=tuple(range(n_params, n_params + n_outs)),
        keep_unused=True)
    sh = NamedSharding(mesh, PartitionSpec("core"))
    zeros = [np.zeros((N_CORES * av.shape[0], *av.shape[1:]), av.dtype)
             for av in out_avals]

    def run(in_maps):
        gin = [jax.device_put(
            np.concatenate([in_maps[c][nm] for c in range(N_CORES)], 0), sh)
            for nm in in_names]
        gz = [jax.device_put(z, sh) for z in zeros]
        out = fn(*gin, *gz)
        got = {nm: np.asarray(o) for nm, o in zip(out_names, out)}
        return [{nm: got[nm].reshape(N_CORES, -1, got[nm].shape[-1])[c]
                 for nm in out_names} for c in range(N_CORES)]

    return run


def _make_in_maps(inputs):
    inp = {k: np.asarray(v) for k, v in inputs.items()}
    concat = np.concatenate([inp["hPrev"], inp["xt"]], axis=0)
    x8 = (concat * S).astype(FP8)
    x16 = concat.astype(F16)
    # AllGather(n, m) gathers the cores' 128-row m-blocks:
    # gathered row 1024*m + 128*c + j  ==  ht row 256*c + 128*m + j
    perm = np.empty(D, np.int64)
    for a in range(NM):
        for c in range(N_CORES):
            base = 1024 * a + 128 * c
            perm[base:base + 128] = np.arange(R * c + 128 * a,
                                              R * c + 128 * a + 128)
    in_maps = []
    for i in range(N_CORES):
        r = slice(i * R, (i + 1) * R)
        m = {"x8": x8, "x16": x16,
             "cprev": np.ascontiguousarray(inp["cPrev"][r]).astype(F16)}
        for g in (1, 2, 4):
            m[f"w{g}t"] = np.ascontiguousarray(inp[f"w{g}"][r].T).astype(FP8)
            m[f"b{g}"] = (inp[f"b{g}"][r] * S).astype(FP8)
        m["w3t"] = np.ascontiguousarray(inp["w3"][r].T).astype(F16)
        m["b3"] = inp["b3"][r].astype(F16)
        m["w5t"] = np.ascontiguousarray(inp["w5"][r].T[perm]).astype(FP8)
        m["b5"] = (inp["b5"][r] * S).astype(F16)
        in_maps.append(m)
    return in_maps


def kernel(**inputs):
    global _CACHE, _RUNNER
    if _CACHE is None:
        _CACHE = _build()
    nc = _CACHE
    in_maps = _make_in_maps(inputs)

    results = None
    if _RUNNER is not False:
        try:
            if _RUNNER is None:
                _RUNNER = _build_runner(nc)
            results = _RUNNER(in_maps)
        except Exception:
            _RUNNER = False  # fall back permanently for this process
    if results is None:
        res = bass_utils.run_bass_kernel_spmd(nc, in_maps,
                                              core_ids=list(range(N_CORES)))
        results = res.results

    ct = np.concatenate([results[i]["ct_o"] for i in range(N_CORES)], 0)
    ht = np.concatenate([results[i]["ht_o"] for i in range(N_CORES)], 0)
    yt = np.concatenate([results[i]["yt_o"] for i in range(N_CORES)], 0)
    return (ct.astype(np.float32), ht.astype(np.float32),
            yt.astype(np.float32))


# revision 3
# speedup vs baseline: 1.1655x; 1.1655x over previous
"""Trainium2 Bass kernel for nn_CustomLstm (D=2048, H=1024), 8-core tensor-parallel.

Sharding: all five weights/biases and outputs are sharded along the units (row)
dimension of W across 8 NeuronCores (256 rows each).  The (D,D) concat
activation is replicated; gate elementwise ops are local; ht is all-gathered
(fp8, one AllGather per 128-row block per 512-col chunk) so the final
w5 @ ht matmul + row softmax is local.

Precision plan (validated vs fp32 reference, worst rel err ~1.0e-2):
  - sigmoid gates w1/w2/w4 and w5: fp8e4m3 DoubleRow matmuls (2 k-chunks per
    instruction, 1.5x measured PE throughput).  x (concat) and ht are scaled
    x64 before the fp8 cast so values stay in e4m3's normal range; the 1/64
    is folded into the activation/exp scale.  b1/b2/b4 ride fp8 (x64).
  - tanh gate w3 (slope 1 passes matmul error straight through): fp16.
  - b3/b5 fp16, cPrev fp16, outputs written fp16 and upcast on host.
"""

import numpy as np
import ml_dtypes

import concourse.bass as bass
import concourse.bacc as bacc
import concourse.mybir as mybir
import concourse.tile as tile
import concourse.bass_utils as bass_utils

BF16 = ml_dtypes.bfloat16
FP8 = ml_dtypes.float8_e4m3
F16 = np.float16

D = 2048          # units == input dim of each weight matrix
N_CORES = 8
R = D // N_CORES  # 256 rows per core
PK = D // 128     # 16 contraction chunks of 128
NN = 4            # 4 column chunks of 512
NCOL = D // NN    # 512
NM = R // 128     # 2 row chunks of 128
S = 64.0          # fp8 input scale

_CACHE = None


def _build(reps=1, single=False, fake_ag=False):
    nc = bacc.Bacc("TRN2", target_bir_lowering=False, debug=False,
                   num_devices=1 if single else N_CORES)
    f32 = mybir.dt.float32
    fp16 = mybir.dt.float16
    fp8 = mybir.dt.float8e4
    AF = mybir.ActivationFunctionType
    DR = mybir.MatmulPerfMode.DoubleRow

    x8_in = nc.dram_tensor("x8", [D, D], fp8, kind="ExternalInput").ap()
    x16_in = nc.dram_tensor("x16", [D, D], fp16, kind="ExternalInput").ap()
    # gate weights transposed: [D(contraction), R]; gate order w1,w2,w3,w4
    wg_in = [nc.dram_tensor(f"w{g}t", [D, R],
                            fp16 if g == 3 else fp8,
                            kind="ExternalInput").ap() for g in range(1, 5)]
    w5_in = nc.dram_tensor("w5t", [D, R], fp8, kind="ExternalInput").ap()
    bg_in = [nc.dram_tensor(f"b{g}", [R, D],
                            fp16 if g == 3 else fp8,
                            kind="ExternalInput").ap() for g in range(1, 5)]
    b5_in = nc.dram_tensor("b5", [R, D], fp16, kind="ExternalInput").ap()
    cprev_in = nc.dram_tensor("cprev", [R, D], fp16, kind="ExternalInput").ap()

    ct_o = nc.dram_tensor("ct_o", [R, D], fp16, kind="ExternalOutput").ap()
    ht_o = nc.dram_tensor("ht_o", [R, D], fp16, kind="ExternalOutput").ap()
    yt_o = nc.dram_tensor("yt_o", [R, D], fp16, kind="ExternalOutput").ap()

    rg = [list(range(N_CORES))]
    GFN = [AF.Sigmoid, AF.Sigmoid, AF.Tanh, AF.Sigmoid]
    GSC = [1.0 / S, 1.0 / S, 1.0, 1.0 / S]   # activation input scale per gate

    with tile.TileContext(nc) as tc:
        with (
            tc.tile_pool(name="wpool", bufs=1) as wpool,
            tc.tile_pool(name="xpool", bufs=2) as xpool,
            tc.tile_pool(name="bpool", bufs=1) as bpool,
            tc.tile_pool(name="hpool", bufs=2) as hpool,
            tc.tile_pool(name="gpool", bufs=1) as gpool,
            tc.tile_pool(name="epool", bufs=1) as epool,
            tc.tile_pool(name="spool", bufs=2) as spool,
            tc.tile_pool(name="ypool", bufs=4) as ypool,
            tc.tile_pool(name="psum", bufs=1, space="PSUM") as pp,
            tc.tile_pool(name="dram", bufs=1, space="DRAM") as dram,
        ):
            for rep in range(reps):
                # ---- resident loads (ACT queue), first-use order ----
                w_sb = [wpool.tile([128, PK * R], fp16 if g == 2 else fp8,
                                   name=f"w{g}sb", tag=f"w{g}sb")
                        for g in range(4)]
                w5_sb = wpool.tile([128, PK * R], fp8, name="w5sb", tag="w5sb")
                b_sb = [bpool.tile([128, NM * D], fp16 if g == 2 else fp8,
                                   name=f"b{g}sb", tag=f"b{g}sb")
                        for g in range(4)]
                b5_sb = bpool.tile([128, NM * D], fp16, name="b5sb", tag="b5sb")
                cp_sb = bpool.tile([128, NM * D], fp16, name="cpsb", tag="cpsb")

                def load_w(dst, src):
                    nc.scalar.dma_start(
                        dst[:].rearrange("p (k m) -> p k m", m=R),
                        src.rearrange("(k p) m -> p k m", p=128))

                def load_b(dst, src):
                    nc.scalar.dma_start(
                        dst[:].rearrange("p (m c) -> p m c", c=D),
                        src.rearrange("(m p) c -> p m c", p=128))

                load_w(w_sb[0], wg_in[0])
                load_w(w_sb[1], wg_in[1])
                load_b(b_sb[0], bg_in[0])
                load_b(b_sb[1], bg_in[1])
                load_w(w_sb[2], wg_in[2])
                load_b(b_sb[2], bg_in[2])
                load_w(w_sb[3], wg_in[3])
                load_b(b_sb[3], bg_in[3])
                load_b(cp_sb, cprev_in)
                load_w(w5_sb, w5_in)
                load_b(b5_sb, b5_in)

                w3d = [w_sb[g][:].rearrange("p (k m) -> p k m", m=R)
                       for g in range(4)]

                ag_out = [None] * NN
                sms = [[None] * NN for _ in range(NM)]
                exs = [epool.tile([128, D], fp16, name=f"ex{m}", tag=f"ex{m}")
                       for m in range(NM)]

                # ---- phase A: gates, ct, ht; AllGather ht per (chunk, m) ----
                for n in range(NN):
                    csl = slice(n * NCOL, (n + 1) * NCOL)
                    x8_sb = xpool.tile([128, PK * NCOL], fp8, name=f"x8_{n}",
                                       tag="x8")
                    nc.sync.dma_start(
                        x8_sb[:].rearrange("p (k c) -> p k c", c=NCOL),
                        x8_in[:, csl].rearrange("(k p) c -> p k c", p=128))
                    x16_sb = xpool.tile([128, PK * NCOL], fp16, name=f"x16_{n}",
                                        tag="x16")
                    nc.sync.dma_start(
                        x16_sb[:].rearrange("p (k c) -> p k c", c=NCOL),
                        x16_in[:, csl].rearrange("(k p) c -> p k c", p=128))
                    x8d = x8_sb[:].rearrange("p (k c) -> p k c", c=NCOL)

                    ctt = gpool.tile([128, NM * NCOL], fp16, name="ctt",
                                     tag="ctt", bufs=2)
                    htt = gpool.tile([128, NM * NCOL], fp16, name="htt",
                                     tag="htt", bufs=2)

                    for m in range(NM):
                        ps = [None] * 4
                        for g in range(4):   # w1, w2, w3, w4
                            p = pp.tile([128, NCOL], f32, name=f"ps{g}",
                                        tag=f"ps{g}")
                            if g == 2:
                                for k in range(PK):
                                    nc.tensor.matmul(
                                        p[:],
                                        w_sb[2][:, k * R + m * 128:
                                                k * R + (m + 1) * 128],
                                        x16_sb[:, k * NCOL:(k + 1) * NCOL],
                                        start=(k == 0), stop=(k == PK - 1))
                            else:
                                for j in range(PK // 2):
                                    nc.tensor.matmul(
                                        p[:],
                                        w3d[g][:, 2 * j:2 * j + 2,
                                               m * 128:(m + 1) * 128],
                                        x8d[:, 2 * j:2 * j + 2, :],
                                        start=(j == 0), stop=(j == PK // 2 - 1),
                                        perf_mode=DR)
                            ps[g] = p

                        acts = []
                        for g in range(4):
                            bsl = b_sb[g][:, m * D + n * NCOL:
                                          m * D + (n + 1) * NCOL]
                            nc.vector.tensor_add(ps[g][:], ps[g][:], bsl)
                            act = gpool.tile([128, NCOL], fp16, name=f"act{g}",
                                             tag=f"act{g}")
                            nc.scalar.activation(act[:], ps[g][:], GFN[g],
                                                 scale=GSC[g])
                            acts.append(act)

                        msl = slice(m * NCOL, (m + 1) * NCOL)
                        cpsl = cp_sb[:, m * D + n * NCOL:m * D + (n + 1) * NCOL]
                        t1 = gpool.tile([128, NCOL], fp16, name="t1", tag="t1")
                        nc.vector.tensor_mul(t1[:], acts[0][:], cpsl)
                        t2 = gpool.tile([128, NCOL], fp16, name="t2", tag="t2")
                        nc.vector.tensor_mul(t2[:], acts[1][:], acts[2][:])
                        nc.vector.tensor_add(ctt[:, msl], t1[:], t2[:])

                        th = gpool.tile([128, NCOL], fp16, name="th", tag="th")
                        nc.scalar.activation(th[:], ctt[:, msl], AF.Tanh)
                        nc.vector.tensor_mul(htt[:, msl], acts[3][:], th[:])
                        htb = gpool.tile([128, NCOL], fp8, name="htb",
                                         tag="htb", bufs=2)
                        nc.scalar.activation(htb[:], htt[:, msl], AF.Copy,
                                             scale=S)

                        if m == 0:
                            agi = dram.tile([R, NCOL], fp8, name=f"agi{n}",
                                            tag=f"agi{n}")
                        nc.scalar.dma_start(agi[m * 128:(m + 1) * 128, :],
                                            htb[:])
                    asp = "Local" if (single or fake_ag) else "Shared"
                    ago = dram.tile([N_CORES * R, NCOL], fp8,
                                    name=f"ago{n}", tag=f"ago{n}",
                                    addr_space=asp)
                    if single or fake_ag:
                        for blk in range(N_CORES):
                            nc.gpsimd.dma_start(
                                ago[blk * R:(blk + 1) * R, :], agi[:])
                    else:
                        nc.gpsimd.collective_compute(
                            "AllGather", mybir.AluOpType.bypass,
                            replica_groups=rg,
                            ins=[agi.opt()], outs=[ago.opt()])
                    ag_out[n] = ago

                    nc.gpsimd.dma_start(
                        ct_o[:, csl].rearrange("(m p) c -> p m c", p=128),
                        ctt[:].rearrange("p (m c) -> p m c", c=NCOL))
                    nc.gpsimd.dma_start(
                        ht_o[:, csl].rearrange("(m p) c -> p m c", p=128),
                        htt[:].rearrange("p (m c) -> p m c", c=NCOL))

                # ---- phase C: z5 = w5 @ ht + b5, row softmax ----
                w53 = w5_sb[:].rearrange("p (k m) -> p k m", m=R)
                for n in range(NN):
                    csl = slice(n * NCOL, (n + 1) * NCOL)
                    h = hpool.tile([128, PK * NCOL], fp8,
                                   name=f"h{n}", tag="h")
                    nc.sync.dma_start(
                        h[:].rearrange("p (k c) -> p k c", c=NCOL),
                        ag_out[n].rearrange("(k p) c -> p k c", p=128))
                    hd = h[:].rearrange("p (k c) -> p k c", c=NCOL)

                    p5 = [pp.tile([128, NCOL], f32, name=f"ps5{m}", tag="ps5",
                                  bufs=2) for m in range(NM)]
                    for m in range(NM):
                        for j in range(PK // 2):
                            nc.tensor.matmul(
                                p5[m][:],
                                w53[:, 2 * j:2 * j + 2,
                                    m * 128:(m + 1) * 128],
                                hd[:, 2 * j:2 * j + 2, :],
                                start=(j == 0), stop=(j == PK // 2 - 1),
                                perf_mode=DR)
                    for m in range(NM):
                        b5sl = b5_sb[:, m * D + n * NCOL:m * D + (n + 1) * NCOL]
                        nc.vector.tensor_add(p5[m][:], p5[m][:], b5sl)
                        sm_t = spool.tile([128, 1], f32, name=f"sm{m}{n}",
                                          tag=f"sm{m}{n}")
                        nc.scalar.activation(exs[m][:, csl], p5[m][:], AF.Exp,
                                             scale=1.0 / S, accum_out=sm_t[:])
                        sms[m][n] = sm_t

                # ---- softmax normalize + yt out ----
                for m in range(NM):
                    s01 = spool.tile([128, 1], f32, name="s01", tag="s01")
                    nc.vector.tensor_add(s01[:], sms[m][0][:], sms[m][1][:])
                    s23 = spool.tile([128, 1], f32, name="s23", tag="s23")
                    nc.vector.tensor_add(s23[:], sms[m][2][:], sms[m][3][:])
                    smt = spool.tile([128, 1], f32, name="smt", tag="smt")
                    nc.vector.tensor_add(smt[:], s01[:], s23[:])
                    rs = spool.tile([128, 1], f32, name="rs", tag="rs")
                    nc.vector.reciprocal(rs[:], smt[:])
                    for j in range(NN):
                        jsl = slice(j * NCOL, (j + 1) * NCOL)
                        yp = ypool.tile([128, NCOL], fp16, name="yp", tag="yp")
                        if j % 2 == 0:
                            nc.vector.tensor_scalar_mul(yp[:], exs[m][:, jsl],
                                                        rs[:])
                        else:
                            nc.scalar.activation(yp[:], exs[m][:, jsl],
                                                 AF.Copy, scale=rs[:])
                        nc.sync.dma_start(yt_o[m * 128:(m + 1) * 128, jsl],
                                          yp[:])

    nc.compile()
    return nc


_RUNNER = None


def _build_runner(nc):
    """Cached jit-compiled SPMD executor mirroring run_bass_kernel_spmd's
    axon/PJRT path, so repeat kernel() calls skip retracing."""
    import jax
    from jax.sharding import Mesh, PartitionSpec, NamedSharding
    from jax.experimental.shard_map import shard_map
    from concourse.bass2jax import (_bass_exec_p, install_neuronx_cc_hook,
                                    partition_id_tensor)

    install_neuronx_cc_hook()
    partition_name = (nc.partition_id_tensor.name
                      if nc.partition_id_tensor else None)
    in_names, out_names, out_avals = [], [], []
    for alloc in nc.m.functions[0].allocations:
        if not isinstance(alloc, mybir.MemoryLocationSet):
            continue
        name = alloc.memorylocations[0].name
        if alloc.kind == "ExternalInput":
            if name != partition_name:
                in_names.append(name)
        elif alloc.kind == "ExternalOutput":
            out_names.append(name)
            out_avals.append(jax.core.ShapedArray(
                tuple(alloc.tensor_shape), mybir.dt.np(alloc.dtype)))
    n_params, n_outs = len(in_names), len(out_names)
    all_in = tuple(in_names + out_names
                   + ([partition_name] if partition_name else []))

    def _body(*args):
        operands = list(args)
        if partition_name is not None:
            operands.append(partition_id_tensor())
        return tuple(_bass_exec_p.bind(
            *operands, out_avals=tuple(out_avals), in_names=all_in,
            out_names=tuple(out_names), lowering_input_output_aliases=(),
            sim_require_finite=True, sim_require_nnan=True, nc=nc))

    devices = jax.devices()[:N_CORES]
    mesh = Mesh(np.asarray(devices), ("core",))
    specs = (PartitionSpec("core"),) * (n_params + n_outs)
    fn = jax.jit(
        shard_map(_body, mesh=mesh, in_specs=specs,
                  out_specs=(PartitionSpec("core"),) * n_outs,
                  check_rep=False),
        donate_argnums=tuple(range(n_params, n_params + n_outs)),
        keep_unused=True)
    sh = NamedSharding(mesh, PartitionSpec("core"))
    zeros = [np.zeros((N_CORES * av.shape[0], *av.shape[1:]), av.dtype)
             for av in out_avals]

    def run(in_maps):
        gin = [jax.device_put(
            np.concatenate([in_maps[c][nm] for c in range(N_CORES)], 0), sh)
            for nm in in_names]
        gz = [jax.device_put(z, sh) for z in zeros]
        out = fn(*gin, *gz)
        got = {nm: np.asarray(o) for nm, o in zip(out_names, out)}
        return [{nm: got[nm].reshape(N_CORES, -1, got[nm].shape[-1])[c]
                 for nm in out_names} for c in range(N_CORES)]

    return run


def _make_in_maps(inputs):
    inp = {k: np.asarray(v) for k, v in inputs.items()}
    concat = np.concatenate([inp["hPrev"], inp["xt"]], axis=0)
    x8 = (concat * S).astype(FP8)
    x16 = concat.astype(F16)
    in_maps = []
    for i in range(N_CORES):
        r = slice(i * R, (i + 1) * R)
        m = {"x8": x8, "x16": x16,
             "cprev": np.ascontiguousarray(inp["cPrev"][r]).astype(F16)}
        for g in (1, 2, 4):
            m[f"w{g}t"] = np.ascontiguousarray(inp[f"w{g}"][r].T).astype(FP8)
            m[f"b{g}"] = (inp[f"b{g}"][r] * S).astype(FP8)
        m["w3t"] = np.ascontiguousarray(inp["w3"][r].T).astype(F16)
        m["b3"] = inp["b3"][r].astype(F16)
        m["w5t"] = np.ascontiguousarray(inp["w5"][r].T).astype(FP8)
        m["b5"] = (inp["b5"][r] * S).astype(F16)
        in_maps.append(m)
    return in_maps


def kernel(**inputs):
    global _CACHE, _RUNNER
    if _CACHE is None:
        _CACHE = _build()
    nc = _CACHE
    in_maps = _make_in_maps(inputs)

    results = None
    if _RUNNER is not False:
        try:
            if _RUNNER is None:
                _RUNNER = _build_runner(nc)
            results = _RUNNER(in_maps)
        except Exception:
            _RUNNER = False  # fall back permanently for this process
    if results is None:
        res = bass_utils.run_bass_kernel_spmd(nc, in_maps,
                                              core_ids=list(range(N_CORES)))
        results = res.results

    ct = np.concatenate([results[i]["ct_o"] for i in range(N_CORES)], 0)
    ht = np.concatenate([results[i]["ht_o"] for i in range(N_CORES)], 0)
    yt = np.concatenate([results[i]["yt_o"] for i in range(N_CORES)], 0)
    return (ct.astype(np.float32), ht.astype(np.float32),
            yt.astype(np.float32))
